# revision 50
# baseline (speedup 1.0000x reference)
"""DeepSets segment-reduce kernel for 8x TRN2 NeuronCores (Bass/Tile).

Computes: out = rho_mlp(segment_mean(phi_mlp(ins), batch))  for
sorted segment ids `batch` in [0, 50000), ins [1M, 128] f32.

v3 design (vs the padded-window baseline):
  - Host REBALANCES segments into windows of <=SEGS_W segs AND <=ROWS_W
    rows (first-fit-decreasing), so every window is exactly NB 128-row
    blocks with ~1.5% padding (baseline: 10%) and every core runs an
    IDENTICAL instruction stream (SPMD-safe static shapes).
  - The one-hot scatter matrix S is [128, 64] per block (segment slot
    within the window), 4x smaller than the baseline's [128, 128], and
    the reduce matmul streams only 64 columns.
  - 1/count is NOT folded into X. It is applied at the PSUM drain of the
    segment sums: t_sb = tps * invc128 (DVE tensor_tensor mult), which
    replaces the plain copy at zero extra cost. X carries only +u0
    (exact phi-b1 absorption: u0 = W1f^-T b1).
  - phi_b2 / rho biases are applied as per-partition ACT biases in the
    feature-major tail; the b2*nz masking matmul is gone: empty segments
    are fixed up on the host (out[empty] = rho_mlp(0)).
  - fp16 output DMA (host converts to fp32).

kernel(**inputs) takes the full unsharded inputs and returns the full
[50000, 128] fp32 output.
"""

import numpy as np
import ml_dtypes

import concourse.mybir as mybir
import concourse.tile as tile
from concourse import bacc
from concourse.bass_utils import run_bass_kernel_spmd

P = 128
N_CORES = 8
NSEG = 50000
SEGS_W = 64          # max segments per window (= S columns, psum slot width)
ROWS_W = 1024        # max rows per window = NB * 128
NB = ROWS_W // P     # blocks per window (8)
GW = 8               # windows per tail group (8*64 = 512 psum cols)
F16 = mybir.dt.float16
F32 = mybir.dt.float32
F8 = mybir.dt.float8e4
FP8NP = ml_dtypes.float8_e4m3

H1R_DT = F16         # h1r SBUF dtype (pocket: mybir.dt.float8e3)
H1R_NP = np.float16


def _f16(a):
    return np.asarray(a, dtype=np.float32).astype(np.float16)


def _pack_windows(counts):
    """First-fit-decreasing: segments -> windows of <=SEGS_W segs,
    <=ROWS_W rows. Returns (n_win, win_of_seg, windows list of seg lists)."""
    nseg = len(counts)
    occupied = np.flatnonzero(counts > 0)
    order = occupied[np.argsort(-counts[occupied], kind="stable")]
    total = int(counts.sum())
    n_min = max(-(-total // ROWS_W), -(-len(occupied) // SEGS_W))
    n_win = -(-(n_min + max(2, n_min // 120)) // N_CORES) * N_CORES
    while True:
        wrows = np.zeros(n_win, np.int64)
        wsegs = np.zeros(n_win, np.int64)
        win_of = np.full(nseg, -1, np.int64)
        avail = list(range(n_win))
        ptr = 0
        ok = True
        for s in order:
            c = counts[s]
            placed = False
            for t in range(len(avail)):
                wi = avail[(ptr + t) % len(avail)]
                if wrows[wi] + c <= ROWS_W and wsegs[wi] < SEGS_W:
                    wrows[wi] += c
                    wsegs[wi] += 1
                    win_of[s] = wi
                    ptr = (ptr + t) % len(avail)
                    if wrows[wi] > ROWS_W - 3 or wsegs[wi] >= SEGS_W:
                        avail.remove(wi)
                        ptr = ptr % max(1, len(avail))
                    else:
                        ptr = (ptr + 1) % len(avail)
                    placed = True
                    break
            if not placed or not avail:
                ok = placed
                break
        if ok:
            return n_win, win_of
        n_win += N_CORES


def _host_prep(ins, batch, wts):
    batch = np.asarray(batch).astype(np.int64)
    ins = np.asarray(ins, dtype=np.float32)
    counts = np.bincount(batch, minlength=NSEG)
    seg_start = np.zeros(NSEG + 1, np.int64)
    np.cumsum(counts, out=seg_start[1:])

    # exact phi-b1 absorption: (x + u0) @ W1f == x @ W1f + b1
    W1f = _f16(wts["phi_W1"]).astype(np.float64)
    b1d = np.asarray(wts["phi_b1"], np.float64)
    u0 = np.linalg.solve(W1f.T, b1d)
    assert np.isfinite(u0).all() and np.abs(u0).max() < 64.0
    ins_u = (ins + u0.astype(np.float32)).astype(np.float16)

    n_win, win_of = _pack_windows(counts)
    nwc = n_win // N_CORES

    # segment -> slot within window: order segs by (window, seg id)
    occ = np.flatnonzero(counts > 0)
    ow = win_of[occ]
    order2 = np.lexsort((occ, ow))
    segs_sorted = occ[order2]
    wins_sorted = ow[order2]
    first = np.r_[True, wins_sorted[1:] != wins_sorted[:-1]]
    slot = np.arange(len(segs_sorted)) - np.maximum.accumulate(
        np.where(first, np.arange(len(segs_sorted)), 0))
    slot_of = np.full(NSEG, -1, np.int64)
    slot_of[segs_sorted] = slot

    invc = np.zeros(NSEG, np.float64)
    invc[occ] = 1.0 / counts[occ]

    ncols = nwc * ROWS_W                        # xt columns per core
    nblk = nwc * NB
    per_core = []
    for c in range(N_CORES):
        rows_pad = np.full(ncols, -1, np.int64)
        m = (wins_sorted >= c * nwc) & (wins_sorted < (c + 1) * nwc)
        segs_c = segs_sorted[m]
        wins_c = wins_sorted[m] - c * nwc
        # rows of each seg are contiguous [seg_start[s], seg_start[s+1])
        cnt_c = counts[segs_c]
        woff = wins_c * ROWS_W
        # in-window offset = cumsum of counts within the window
        ccs = np.cumsum(cnt_c) - cnt_c
        wstart = np.zeros(len(segs_c), np.int64)
        if len(segs_c):
            fw = np.r_[True, wins_c[1:] != wins_c[:-1]]
            base = np.where(fw, ccs, 0)
            wstart = ccs - np.maximum.accumulate(base)
        dst0 = woff + wstart                    # first xt col of each seg
        starts = seg_start[segs_c]
        flat_dst = np.repeat(dst0, cnt_c) + (
            np.arange(int(cnt_c.sum()))
            - np.repeat(np.cumsum(cnt_c) - cnt_c, cnt_c))
        flat_src = np.repeat(starts, cnt_c) + (
            np.arange(int(cnt_c.sum()))
            - np.repeat(np.cumsum(cnt_c) - cnt_c, cnt_c))
        rows_pad[flat_dst] = flat_src

        valid = rows_pad >= 0
        xtb = np.zeros((ncols, P), np.float16)
        xtb[valid] = ins_u[rows_pad[valid]]
        xt = np.ascontiguousarray(xtb.T)

        # one-hot S fp8 [128, nblk*64]
        s8 = np.zeros((P, nblk * SEGS_W), np.uint8)
        ci = np.flatnonzero(valid)
        segcol = slot_of[batch[rows_pad[ci]]]
        blk = ci // P
        s8[ci % P, blk * SEGS_W + segcol] = 1
        sfp8 = s8.astype(FP8NP)

        # invc128 fp16 [128, nwc*64] (replicated across partitions)
        iv = np.zeros(nwc * SEGS_W, np.float32)
        iv[wins_c * SEGS_W + slot_of[segs_c]] = invc[segs_c].astype(np.float32)
        invc128 = np.ascontiguousarray(
            np.broadcast_to(iv.astype(np.float16), (P, nwc * SEGS_W)))

        per_core.append({"xt": xt, "sfp8": sfp8, "invc128": invc128})

    # output slot -> segment mapping
    col_of_seg = np.full(NSEG, -1, np.int64)
    col_of_seg[segs_sorted] = wins_sorted * SEGS_W + slot_of[segs_sorted]
    return per_core, nwc, col_of_seg, counts


def _host_consts(wts):
    cpack16 = np.concatenate(
        [_f16(wts["phi_W1"]), _f16(wts["phi_W2"]),
         _f16(wts["rho_W1"]), _f16(wts["rho_W2"])], axis=1)
    cpack32 = np.stack(
        [np.asarray(wts["phi_b2"], np.float32),
         np.asarray(wts["rho_b1"], np.float32),
         np.asarray(wts["rho_b2"], np.float32)], axis=1)
    return {"cpack16": cpack16, "cpack32": cpack32}


def _build(nwc, consts_np):
    """Emit the SPMD single-core program (same NEFF for all 8 cores)."""
    nblk = nwc * NB
    nc = bacc.Bacc("TRN2", target_bir_lowering=False, debug=False,
                   num_devices=N_CORES)

    d_xt = nc.dram_tensor("xt", [P, nwc * ROWS_W], F16,
                          kind="ExternalInput").ap()
    d_s = nc.dram_tensor("sfp8", [P, nblk * SEGS_W], F8,
                         kind="ExternalInput").ap()
    d_iv = nc.dram_tensor("invc128", [P, nwc * SEGS_W], F16,
                          kind="ExternalInput").ap()
    d_consts = {
        k: nc.dram_tensor(
            k, list(v.shape), mybir.dt.from_np(v.dtype), kind="ExternalInput"
        ).ap()
        for k, v in consts_np.items()
    }
    d_out = nc.dram_tensor("outT", [P, nwc * SEGS_W], F16,
                           kind="ExternalOutput").ap()

    groups = []
    w0 = 0
    while w0 < nwc:
        groups.append((w0, min(GW, nwc - w0)))
        w0 += GW

    with tile.TileContext(nc) as tc:
        with (
            tc.tile_pool(name="const", bufs=1) as constp,
            tc.tile_pool(name="outsb", bufs=1) as outp,
            tc.tile_pool(name="xt", bufs=9) as xtp,
            tc.tile_pool(name="sfp", bufs=3) as sfpp,
            tc.tile_pool(name="h1r", bufs=5) as h1rp,
            tc.tile_pool(name="tail16", bufs=6) as tailp,
            tc.tile_pool(name="h1ps", bufs=4, space="PSUM") as h1psp,
            tc.tile_pool(name="tps", bufs=2, space="PSUM") as tpsp,
            tc.tile_pool(name="tailps", bufs=2, space="PSUM") as tailpsp,
        ):
            cs_ = {}
            for k, v in consts_np.items():
                cs_[k] = constp.tile(
                    list(v.shape), mybir.dt.from_np(v.dtype), name=f"c_{k}")
                nc.scalar.dma_start(cs_[k], d_consts[k])
            w1_c = cs_["cpack16"][:, 0:128]
            w2_c = cs_["cpack16"][:, 128:256]
            rw1_c = cs_["cpack16"][:, 256:384]
            rw2_c = cs_["cpack16"][:, 384:512]
            b2_c = cs_["cpack32"][:, 0:1]
            rb1_c = cs_["cpack32"][:, 1:2]
            rb2_c = cs_["cpack32"][:, 2:3]
            # ivsb (2MB) goes on the striped sync ring, issued lazily at
            # window 3 so it doesn't delay the startup xt/S quarters and
            # doesn't serialize on the scalar ring's single queue
            ivsb = constp.tile([P, nwc * SEGS_W], F16, name="ivsb")
            outsb = outp.tile([P, nwc * SEGS_W], F16)

            # DMA granularity: xt per XQ windows, S per SQ windows
            # (both 2-4KB/partition lines)
            XQ, SQ = 2, 16
            XW = XQ * ROWS_W
            SW = SQ * NB * SEGS_W
            xt_tiles, s_tiles = {}, {}

            def fetch_xt(w):
                q = w // XQ
                if q in xt_tiles:
                    return
                t = xtp.tile([P, XW], F16, tag="xt", name=f"xt_{q}")
                lo = q * XW
                hi = min((q + 1) * XW, nwc * ROWS_W)
                if q == 0:
                    # split so the first window's L1 can start sooner
                    nc.sync.dma_start(t[:, :ROWS_W], d_xt[:, :ROWS_W])
                    nc.sync.dma_start(t[:, ROWS_W: hi - lo],
                                      d_xt[:, ROWS_W:hi])
                else:
                    nc.sync.dma_start(t[:, : hi - lo], d_xt[:, lo:hi])
                xt_tiles[q] = t

            def fetch_s(w):
                q = w // SQ
                if q in s_tiles:
                    return
                t = sfpp.tile([P, SW], F8, tag="sfp", name=f"sfp_{q}")
                lo = q * SW
                hi = min((q + 1) * SW, nblk * SEGS_W)
                wS = NB * SEGS_W
                if q == 0:
                    # split so the first window's reduce can start sooner
                    nc.sync.dma_start(t[:, :wS], d_s[:, :wS])
                    nc.sync.dma_start(t[:, wS: hi - lo], d_s[:, wS:hi])
                else:
                    nc.sync.dma_start(t[:, : hi - lo], d_s[:, lo:hi])
                s_tiles[q] = t

            drain_flip = [0]
            h1r_tiles = {}
            tps_tiles = {}

            PF = 14   # DMA prefetch depth in windows

            def emit_l1(w):
                """xt DMA + phi layer 1 + relu drain for window w."""
                for pw in range(w, min(w + PF + 1, nwc)):
                    fetch_xt(pw)
                    fetch_s(pw)
                xt = xt_tiles[w // XQ]
                xoff = (w % XQ) * ROWS_W
                h1r = h1rp.tile([P, ROWS_W], H1R_DT, tag="h1r", name=f"h1r_{w}")
                for cj in range(NB // 4):
                    h1ps = h1psp.tile([P, 512], F32, space="PSUM",
                                      tag="h1ps")
                    for j in range(4):
                        nc.tensor.matmul(
                            h1ps[:, j * P: (j + 1) * P],
                            lhsT=xt[:, xoff + (cj * 4 + j) * P:
                                    xoff + (cj * 4 + j + 1) * P],
                            rhs=w1_c, start=True, stop=(j == 3),
                        )
                    dst = h1r[:, cj * 512: (cj + 1) * 512]
                    if drain_flip[0] & 1:
                        nc.scalar.activation(
                            dst, h1ps, mybir.ActivationFunctionType.Relu)
                    else:
                        nc.vector.tensor_scalar(
                            dst, h1ps, 0.0, None, op0=mybir.AluOpType.max)
                    drain_flip[0] += 1
                h1r_tiles[w] = h1r

            def emit_reduce(w):
                """segment-sum reduce of window w into its group psum."""
                g = w // GW
                if g not in tps_tiles:
                    tps_tiles[g] = tpsp.tile(
                        [P, GW * SEGS_W], F32, space="PSUM", tag="tps",
                        name=f"tps_{g}")
                tps = tps_tiles[g]
                st = s_tiles[w // SQ]
                soff = (w % SQ) * NB * SEGS_W
                h1r = h1r_tiles.pop(w)
                reg = tps[:, (w % GW) * SEGS_W: (w % GW + 1) * SEGS_W]
                for b in range(NB):
                    nc.tensor.matmul(
                        reg,
                        lhsT=h1r[:, b * P: (b + 1) * P],
                        rhs=st[:, soff + b * SEGS_W:
                               soff + (b + 1) * SEGS_W],
                        start=(b == 0), stop=(b == NB - 1),
                    )

            # staged tail: copies go at window START (ahead of relu in the
            # strict ACT/DVE FIFOs), matmuls at window END (after reduce),
            # each op a full window after its input was produced
            tail_state = {}

            def t_tsb(g0, gn):
                tps = tps_tiles.pop(g0 // GW)
                gc = gn * SEGS_W
                t_sb = tailp.tile([P, gc], F16, tag="t_sb",
                                  padded_shape=[P, GW * SEGS_W],
                                  name=f"t_sb_{g0}")
                nc.vector.tensor_tensor(
                    t_sb, tps[:, :gc],
                    ivsb[:, g0 * SEGS_W: g0 * SEGS_W + gc],
                    mybir.AluOpType.mult)
                tail_state[g0] = t_sb

            def t_smmm(g0, gn):
                gc = gn * SEGS_W
                smps = tailpsp.tile([P, gc], F32, space="PSUM",
                                    tag="tailps",
                                    padded_shape=[P, GW * SEGS_W],
                                    name=f"smps_{g0}")
                nc.tensor.matmul(smps, lhsT=w2_c, rhs=tail_state.pop(g0),
                                 start=True, stop=True)
                tail_state[g0] = smps

            def t_smcp(g0, gn):
                gc = gn * SEGS_W
                sm_sb = tailp.tile([P, gc], F16, tag="sm_sb",
                                   padded_shape=[P, GW * SEGS_W],
                                   name=f"sm_sb_{g0}")
                nc.scalar.activation(
                    sm_sb, tail_state.pop(g0),
                    mybir.ActivationFunctionType.Identity, bias=b2_c)
                tail_state[g0] = sm_sb

            def t_r1mm(g0, gn):
                gc = gn * SEGS_W
                r1ps = tailpsp.tile([P, gc], F32, space="PSUM",
                                    tag="tailps",
                                    padded_shape=[P, GW * SEGS_W],
                                    name=f"r1ps_{g0}")
                nc.tensor.matmul(r1ps, lhsT=rw1_c, rhs=tail_state.pop(g0),
                                 start=True, stop=True)
                tail_state[g0] = r1ps

            def t_r1cp(g0, gn):
                gc = gn * SEGS_W
                r1_sb = tailp.tile([P, gc], F16, tag="r1_sb",
                                   padded_shape=[P, GW * SEGS_W],
                                   name=f"r1_sb_{g0}")
                nc.vector.tensor_scalar(
                    r1_sb, tail_state.pop(g0), rb1_c, 0.0,
                    op0=mybir.AluOpType.add, op1=mybir.AluOpType.max)
                tail_state[g0] = r1_sb

            def t_opmm(g0, gn):
                gc = gn * SEGS_W
                ops_ = tailpsp.tile([P, gc], F32, space="PSUM",
                                    tag="tailps",
                                    padded_shape=[P, GW * SEGS_W],
                                    name=f"ops_{g0}")
                nc.tensor.matmul(ops_, lhsT=rw2_c, rhs=tail_state.pop(g0),
                                 start=True, stop=True)
                tail_state[g0] = ops_

            def t_outcp(g0, gn):
                gc = gn * SEGS_W
                nc.scalar.activation(
                    outsb[:, g0 * SEGS_W: g0 * SEGS_W + gc],
                    tail_state.pop(g0),
                    mybir.ActivationFunctionType.Identity, bias=rb2_c)
                nc.sync.dma_start(
                    d_out[:, g0 * SEGS_W: g0 * SEGS_W + gc],
                    outsb[:, g0 * SEGS_W: g0 * SEGS_W + gc])

            TAIL_PRE = {1: t_tsb, 3: t_smcp, 5: t_r1cp, 7: t_outcp}
            TAIL_POST = {2: t_smmm, 4: t_r1mm, 6: t_opmm}

            # software pipeline: L1 runs LOOKAHEAD windows ahead of the
            # reduce so the relu drains never stall the PE queue
            LOOKAHEAD = 2
            pre_at, post_at = {}, {}
            for w in range(nwc + LOOKAHEAD + 8):
                if w == 3:
                    nc.sync.dma_start(ivsb, d_iv)
                for fn, g0, gn in pre_at.pop(w, []):
                    fn(g0, gn)
                if w < nwc:
                    emit_l1(w)
                r = w - LOOKAHEAD
                if 0 <= r < nwc:
                    emit_reduce(r)
                    if r % GW == GW - 1 or r == nwc - 1:
                        g0 = (r // GW) * GW
                        gn = r - g0 + 1
                        dly = 5 if g0 == 0 else 0
                        for k, fn in TAIL_PRE.items():
                            pre_at.setdefault(w + k + dly, []).append(
                                (fn, g0, gn))
                        for k, fn in TAIL_POST.items():
                            post_at.setdefault(w + k + dly, []).append(
                                (fn, g0, gn))
                for fn, g0, gn in post_at.pop(w, []):
                    fn(g0, gn)

    nc.compile()
    return nc


def _rho0(wts):
    """rho_mlp(0) for empty segments (host fp32)."""
    r1 = np.maximum(np.asarray(wts["rho_b1"], np.float64), 0.0)
    return (r1 @ np.asarray(wts["rho_W2"], np.float64)
            + np.asarray(wts["rho_b2"], np.float64)).astype(np.float32)


def _run(inputs, trace=False, **hw_kwargs):
    ins = np.asarray(inputs["ins"])
    batch = np.asarray(inputs["batch"])
    per_core, nwc, col_of_seg, counts = _host_prep(ins, batch, inputs)
    consts_np = _host_consts(inputs)
    nc = _build(nwc, consts_np)

    in_maps = []
    for c in range(N_CORES):
        m = dict(consts_np)
        m.update(per_core[c])
        in_maps.append(m)
    res = run_bass_kernel_spmd(
        nc, in_maps, core_ids=list(range(N_CORES)), trace=trace, **hw_kwargs
    )
    outs = [np.asarray(r["outT"], np.float32) for r in res.results]
    allc = np.concatenate(outs, axis=1)          # [128, n_win*64]
    full = np.empty((NSEG, P), np.float32)
    occ = counts > 0
    full[occ] = allc[:, col_of_seg[occ]].T
    if (~occ).any():
        full[~occ] = _rho0(inputs)
    return full, res


def kernel(**inputs):
    out, _ = _run(inputs)
    return out


# revision 51
# speedup vs baseline: 1.2353x; 1.2353x over previous
"""DeepSets segment-reduce kernel for 8x TRN2 NeuronCores (Bass/Tile).

Computes: out = rho_mlp(segment_mean(phi_mlp(ins), batch))  for
sorted segment ids `batch` in [0, 50000), ins [1M, 128] f32.

v3 design (vs the padded-window baseline):
  - Host REBALANCES segments into windows of <=SEGS_W segs AND <=ROWS_W
    rows (first-fit-decreasing), so every window is exactly NB 128-row
    blocks with ~1.5% padding (baseline: 10%) and every core runs an
    IDENTICAL instruction stream (SPMD-safe static shapes).
  - The one-hot scatter matrix S is [128, 64] per block (segment slot
    within the window), 4x smaller than the baseline's [128, 128], and
    the reduce matmul streams only 64 columns.
  - 1/count is NOT folded into X. It is applied at the PSUM drain of the
    segment sums: t_sb = tps * invc128 (DVE tensor_tensor mult), which
    replaces the plain copy at zero extra cost. X carries only +u0
    (exact phi-b1 absorption: u0 = W1f^-T b1).
  - phi_b2 / rho biases are applied as per-partition ACT biases in the
    feature-major tail; the b2*nz masking matmul is gone: empty segments
    are fixed up on the host (out[empty] = rho_mlp(0)).
  - fp16 output DMA (host converts to fp32).

kernel(**inputs) takes the full unsharded inputs and returns the full
[50000, 128] fp32 output.
"""

import numpy as np
import ml_dtypes

import concourse.mybir as mybir
import concourse.tile as tile
from concourse import bacc
from concourse.bass_utils import run_bass_kernel_spmd

P = 128
N_CORES = 8
NSEG = 50000
SEGS_W = 64          # max segments per window (= S columns, psum slot width)
ROWS_W = 1024        # max rows per window = NB * 128
NB = ROWS_W // P     # blocks per window (8)
GW = 8               # windows per tail group (8*64 = 512 psum cols)
F16 = mybir.dt.float16
F32 = mybir.dt.float32
F8 = mybir.dt.float8e4
FP8NP = ml_dtypes.float8_e4m3

H1R_DT = F16         # h1r SBUF dtype (pocket: mybir.dt.float8e3)
H1R_NP = np.float16


def _f16(a):
    return np.asarray(a, dtype=np.float32).astype(np.float16)


def _pack_windows(counts):
    """First-fit-decreasing: segments -> windows of <=SEGS_W segs,
    <=ROWS_W rows. Returns (n_win, win_of_seg, windows list of seg lists)."""
    nseg = len(counts)
    occupied = np.flatnonzero(counts > 0)
    order = occupied[np.argsort(-counts[occupied], kind="stable")]
    total = int(counts.sum())
    n_min = max(-(-total // ROWS_W), -(-len(occupied) // SEGS_W))
    n_win = -(-(n_min + max(2, n_min // 120)) // N_CORES) * N_CORES
    while True:
        wrows = np.zeros(n_win, np.int64)
        wsegs = np.zeros(n_win, np.int64)
        win_of = np.full(nseg, -1, np.int64)
        avail = list(range(n_win))
        ptr = 0
        ok = True
        for s in order:
            c = counts[s]
            placed = False
            for t in range(len(avail)):
                wi = avail[(ptr + t) % len(avail)]
                if wrows[wi] + c <= ROWS_W and wsegs[wi] < SEGS_W:
                    wrows[wi] += c
                    wsegs[wi] += 1
                    win_of[s] = wi
                    ptr = (ptr + t) % len(avail)
                    if wrows[wi] > ROWS_W - 3 or wsegs[wi] >= SEGS_W:
                        avail.remove(wi)
                        ptr = ptr % max(1, len(avail))
                    else:
                        ptr = (ptr + 1) % len(avail)
                    placed = True
                    break
            if not placed or not avail:
                ok = placed
                break
        if ok:
            return n_win, win_of
        n_win += N_CORES


def _host_prep(ins, batch, wts):
    batch = np.asarray(batch).astype(np.int64)
    ins = np.asarray(ins, dtype=np.float32)
    counts = np.bincount(batch, minlength=NSEG)
    seg_start = np.zeros(NSEG + 1, np.int64)
    np.cumsum(counts, out=seg_start[1:])

    # exact phi-b1 absorption: (x + u0) @ W1f == x @ W1f + b1
    W1f = _f16(wts["phi_W1"]).astype(np.float64)
    b1d = np.asarray(wts["phi_b1"], np.float64)
    u0 = np.linalg.solve(W1f.T, b1d)
    assert np.isfinite(u0).all() and np.abs(u0).max() < 64.0
    ins_u = (ins + u0.astype(np.float32)).astype(np.float16)

    n_win, win_of = _pack_windows(counts)
    nwc = n_win // N_CORES

    # segment -> slot within window: order segs by (window, seg id)
    occ = np.flatnonzero(counts > 0)
    ow = win_of[occ]
    order2 = np.lexsort((occ, ow))
    segs_sorted = occ[order2]
    wins_sorted = ow[order2]
    first = np.r_[True, wins_sorted[1:] != wins_sorted[:-1]]
    slot = np.arange(len(segs_sorted)) - np.maximum.accumulate(
        np.where(first, np.arange(len(segs_sorted)), 0))
    slot_of = np.full(NSEG, -1, np.int64)
    slot_of[segs_sorted] = slot

    invc = np.zeros(NSEG, np.float64)
    invc[occ] = 1.0 / counts[occ]

    ncols = nwc * ROWS_W                        # xt columns per core
    nblk = nwc * NB
    per_core = []
    for c in range(N_CORES):
        rows_pad = np.full(ncols, -1, np.int64)
        m = (wins_sorted >= c * nwc) & (wins_sorted < (c + 1) * nwc)
        segs_c = segs_sorted[m]
        wins_c = wins_sorted[m] - c * nwc
        # rows of each seg are contiguous [seg_start[s], seg_start[s+1])
        cnt_c = counts[segs_c]
        woff = wins_c * ROWS_W
        # in-window offset = cumsum of counts within the window
        ccs = np.cumsum(cnt_c) - cnt_c
        wstart = np.zeros(len(segs_c), np.int64)
        if len(segs_c):
            fw = np.r_[True, wins_c[1:] != wins_c[:-1]]
            base = np.where(fw, ccs, 0)
            wstart = ccs - np.maximum.accumulate(base)
        dst0 = woff + wstart                    # first xt col of each seg
        starts = seg_start[segs_c]
        flat_dst = np.repeat(dst0, cnt_c) + (
            np.arange(int(cnt_c.sum()))
            - np.repeat(np.cumsum(cnt_c) - cnt_c, cnt_c))
        flat_src = np.repeat(starts, cnt_c) + (
            np.arange(int(cnt_c.sum()))
            - np.repeat(np.cumsum(cnt_c) - cnt_c, cnt_c))
        rows_pad[flat_dst] = flat_src

        valid = rows_pad >= 0
        xtb = np.zeros((ncols, P), np.float16)
        xtb[valid] = ins_u[rows_pad[valid]]
        xt = np.ascontiguousarray(xtb.T)

        # one-hot S fp8 [128, nblk*64]
        s8 = np.zeros((P, nblk * SEGS_W), np.uint8)
        ci = np.flatnonzero(valid)
        segcol = slot_of[batch[rows_pad[ci]]]
        blk = ci // P
        s8[ci % P, blk * SEGS_W + segcol] = 1
        sfp8 = s8.astype(FP8NP)

        # invc128 fp16 [128, nwc*64] (replicated across partitions)
        iv = np.zeros(nwc * SEGS_W, np.float32)
        iv[wins_c * SEGS_W + slot_of[segs_c]] = invc[segs_c].astype(np.float32)
        invc128 = np.ascontiguousarray(
            np.broadcast_to(iv.astype(np.float16), (P, nwc * SEGS_W)))

        per_core.append({"xt": xt, "sfp8": sfp8, "invc128": invc128})

    # output slot -> segment mapping
    col_of_seg = np.full(NSEG, -1, np.int64)
    col_of_seg[segs_sorted] = wins_sorted * SEGS_W + slot_of[segs_sorted]
    return per_core, nwc, col_of_seg, counts


def _host_consts(wts):
    cpack16 = np.concatenate(
        [_f16(wts["phi_W1"]), _f16(wts["phi_W2"]),
         _f16(wts["rho_W1"]), _f16(wts["rho_W2"])], axis=1)
    cpack32 = np.stack(
        [np.asarray(wts["phi_b2"], np.float32),
         np.asarray(wts["rho_b1"], np.float32),
         np.asarray(wts["rho_b2"], np.float32)], axis=1)
    return {"cpack16": cpack16, "cpack32": cpack32}


def _build(nwc, consts_np):
    """Emit the SPMD single-core program (same NEFF for all 8 cores)."""
    nblk = nwc * NB
    nc = bacc.Bacc("TRN2", target_bir_lowering=False, debug=False,
                   num_devices=N_CORES)

    d_xt = nc.dram_tensor("xt", [P, nwc * ROWS_W], F16,
                          kind="ExternalInput").ap()
    d_s = nc.dram_tensor("sfp8", [P, nblk * SEGS_W], F8,
                         kind="ExternalInput").ap()
    d_iv = nc.dram_tensor("invc128", [P, nwc * SEGS_W], F16,
                          kind="ExternalInput").ap()
    d_consts = {
        k: nc.dram_tensor(
            k, list(v.shape), mybir.dt.from_np(v.dtype), kind="ExternalInput"
        ).ap()
        for k, v in consts_np.items()
    }
    d_out = nc.dram_tensor("outT", [P, nwc * SEGS_W], F16,
                           kind="ExternalOutput").ap()

    groups = []
    w0 = 0
    while w0 < nwc:
        groups.append((w0, min(GW, nwc - w0)))
        w0 += GW

    with tile.TileContext(nc) as tc:
        with (
            tc.tile_pool(name="const", bufs=1) as constp,
            tc.tile_pool(name="outsb", bufs=1) as outp,
            tc.tile_pool(name="xt", bufs=9) as xtp,
            tc.tile_pool(name="sfp", bufs=3) as sfpp,
            tc.tile_pool(name="h1r", bufs=5) as h1rp,
            tc.tile_pool(name="tail16", bufs=6) as tailp,
            tc.tile_pool(name="h1ps", bufs=4, space="PSUM") as h1psp,
            tc.tile_pool(name="tps", bufs=2, space="PSUM") as tpsp,
            tc.tile_pool(name="tailps", bufs=2, space="PSUM") as tailpsp,
        ):
            cs_ = {}
            for k, v in consts_np.items():
                cs_[k] = constp.tile(
                    list(v.shape), mybir.dt.from_np(v.dtype), name=f"c_{k}")
                nc.scalar.dma_start(cs_[k], d_consts[k])
            w1_c = cs_["cpack16"][:, 0:128]
            w2_c = cs_["cpack16"][:, 128:256]
            rw1_c = cs_["cpack16"][:, 256:384]
            rw2_c = cs_["cpack16"][:, 384:512]
            b2_c = cs_["cpack32"][:, 0:1]
            rb1_c = cs_["cpack32"][:, 1:2]
            rb2_c = cs_["cpack32"][:, 2:3]
            # ivsb (2MB) goes on the striped sync ring, issued lazily at
            # window 3 so it doesn't delay the startup xt/S quarters and
            # doesn't serialize on the scalar ring's single queue
            ivsb = constp.tile([P, nwc * SEGS_W], F16, name="ivsb")
            outsb = outp.tile([P, nwc * SEGS_W], F16)

            # DMA granularity: xt per XQ windows, S per SQ windows
            # (both 2-4KB/partition lines)
            XQ, SQ = 2, 8
            XW = XQ * ROWS_W
            SW = SQ * NB * SEGS_W
            xt_tiles, s_tiles = {}, {}

            def fetch_xt(w):
                q = w // XQ
                if q in xt_tiles:
                    return
                t = xtp.tile([P, XW], F16, tag="xt", name=f"xt_{q}")
                lo = q * XW
                hi = min((q + 1) * XW, nwc * ROWS_W)
                if q == 0:
                    # split so the first window's L1 can start sooner
                    nc.sync.dma_start(t[:, :ROWS_W], d_xt[:, :ROWS_W])
                    nc.sync.dma_start(t[:, ROWS_W: hi - lo],
                                      d_xt[:, ROWS_W:hi])
                else:
                    nc.sync.dma_start(t[:, : hi - lo], d_xt[:, lo:hi])
                xt_tiles[q] = t

            def fetch_s(w):
                q = w // SQ
                if q in s_tiles:
                    return
                t = sfpp.tile([P, SW], F8, tag="sfp", name=f"sfp_{q}")
                lo = q * SW
                hi = min((q + 1) * SW, nblk * SEGS_W)
                wS = NB * SEGS_W
                if q == 0:
                    # split so the first window's reduce can start sooner
                    nc.sync.dma_start(t[:, :wS], d_s[:, :wS])
                    nc.sync.dma_start(t[:, wS: hi - lo], d_s[:, wS:hi])
                else:
                    nc.sync.dma_start(t[:, : hi - lo], d_s[:, lo:hi])
                s_tiles[q] = t

            drain_flip = [0]
            h1r_tiles = {}
            tps_tiles = {}

            PF = 14   # DMA prefetch depth in windows

            def emit_l1(w):
                """xt DMA + phi layer 1 + relu drain for window w."""
                for pw in range(w, min(w + PF + 1, nwc)):
                    fetch_xt(pw)
                    fetch_s(pw)
                xt = xt_tiles[w // XQ]
                xoff = (w % XQ) * ROWS_W
                h1r = h1rp.tile([P, ROWS_W], H1R_DT, tag="h1r", name=f"h1r_{w}")
                for cj in range(NB // 4):
                    h1ps = h1psp.tile([P, 512], F32, space="PSUM",
                                      tag="h1ps")
                    for j in range(4):
                        nc.tensor.matmul(
                            h1ps[:, j * P: (j + 1) * P],
                            lhsT=xt[:, xoff + (cj * 4 + j) * P:
                                    xoff + (cj * 4 + j + 1) * P],
                            rhs=w1_c, start=True, stop=(j == 3),
                        )
                    dst = h1r[:, cj * 512: (cj + 1) * 512]
                    if drain_flip[0] & 1:
                        nc.scalar.activation(
                            dst, h1ps, mybir.ActivationFunctionType.Relu)
                    else:
                        nc.vector.tensor_scalar(
                            dst, h1ps, 0.0, None, op0=mybir.AluOpType.max)
                    drain_flip[0] += 1
                h1r_tiles[w] = h1r

            def emit_reduce(w):
                """segment-sum reduce of window w into its group psum."""
                g = w // GW
                if g not in tps_tiles:
                    tps_tiles[g] = tpsp.tile(
                        [P, GW * SEGS_W], F32, space="PSUM", tag="tps",
                        name=f"tps_{g}")
                tps = tps_tiles[g]
                st = s_tiles[w // SQ]
                soff = (w % SQ) * NB * SEGS_W
                h1r = h1r_tiles.pop(w)
                reg = tps[:, (w % GW) * SEGS_W: (w % GW + 1) * SEGS_W]
                for b in range(NB):
                    nc.tensor.matmul(
                        reg,
                        lhsT=h1r[:, b * P: (b + 1) * P],
                        rhs=st[:, soff + b * SEGS_W:
                               soff + (b + 1) * SEGS_W],
                        start=(b == 0), stop=(b == NB - 1),
                    )

            # staged tail: copies go at window START (ahead of relu in the
            # strict ACT/DVE FIFOs), matmuls at window END (after reduce),
            # each op a full window after its input was produced
            tail_state = {}

            def t_tsb(g0, gn):
                tps = tps_tiles.pop(g0 // GW)
                gc = gn * SEGS_W
                t_sb = tailp.tile([P, gc], F16, tag="t_sb",
                                  padded_shape=[P, GW * SEGS_W],
                                  name=f"t_sb_{g0}")
                nc.vector.tensor_tensor(
                    t_sb, tps[:, :gc],
                    ivsb[:, g0 * SEGS_W: g0 * SEGS_W + gc],
                    mybir.AluOpType.mult)
                tail_state[g0] = t_sb

            def t_smmm(g0, gn):
                gc = gn * SEGS_W
                smps = tailpsp.tile([P, gc], F32, space="PSUM",
                                    tag="tailps",
                                    padded_shape=[P, GW * SEGS_W],
                                    name=f"smps_{g0}")
                nc.tensor.matmul(smps, lhsT=w2_c, rhs=tail_state.pop(g0),
                                 start=True, stop=True)
                tail_state[g0] = smps

            def t_smcp(g0, gn):
                gc = gn * SEGS_W
                sm_sb = tailp.tile([P, gc], F16, tag="sm_sb",
                                   padded_shape=[P, GW * SEGS_W],
                                   name=f"sm_sb_{g0}")
                nc.scalar.activation(
                    sm_sb, tail_state.pop(g0),
                    mybir.ActivationFunctionType.Identity, bias=b2_c)
                tail_state[g0] = sm_sb

            def t_r1mm(g0, gn):
                gc = gn * SEGS_W
                r1ps = tailpsp.tile([P, gc], F32, space="PSUM",
                                    tag="tailps",
                                    padded_shape=[P, GW * SEGS_W],
                                    name=f"r1ps_{g0}")
                nc.tensor.matmul(r1ps, lhsT=rw1_c, rhs=tail_state.pop(g0),
                                 start=True, stop=True)
                tail_state[g0] = r1ps

            def t_r1cp(g0, gn):
                gc = gn * SEGS_W
                r1_sb = tailp.tile([P, gc], F16, tag="r1_sb",
                                   padded_shape=[P, GW * SEGS_W],
                                   name=f"r1_sb_{g0}")
                nc.vector.tensor_scalar(
                    r1_sb, tail_state.pop(g0), rb1_c, 0.0,
                    op0=mybir.AluOpType.add, op1=mybir.AluOpType.max)
                tail_state[g0] = r1_sb

            def t_opmm(g0, gn):
                gc = gn * SEGS_W
                ops_ = tailpsp.tile([P, gc], F32, space="PSUM",
                                    tag="tailps",
                                    padded_shape=[P, GW * SEGS_W],
                                    name=f"ops_{g0}")
                nc.tensor.matmul(ops_, lhsT=rw2_c, rhs=tail_state.pop(g0),
                                 start=True, stop=True)
                tail_state[g0] = ops_

            def t_outcp(g0, gn):
                gc = gn * SEGS_W
                nc.scalar.activation(
                    outsb[:, g0 * SEGS_W: g0 * SEGS_W + gc],
                    tail_state.pop(g0),
                    mybir.ActivationFunctionType.Identity, bias=rb2_c)
                nc.sync.dma_start(
                    d_out[:, g0 * SEGS_W: g0 * SEGS_W + gc],
                    outsb[:, g0 * SEGS_W: g0 * SEGS_W + gc])

            TAIL_PRE = {1: t_tsb, 3: t_smcp, 5: t_r1cp, 7: t_outcp}
            TAIL_POST = {2: t_smmm, 4: t_r1mm, 6: t_opmm}

            # software pipeline: L1 runs LOOKAHEAD windows ahead of the
            # reduce so the relu drains never stall the PE queue
            LOOKAHEAD = 2
            pre_at, post_at = {}, {}
            for w in range(nwc + LOOKAHEAD + 8):
                if w == 3:
                    nc.sync.dma_start(ivsb, d_iv)
                for fn, g0, gn in pre_at.pop(w, []):
                    fn(g0, gn)
                if w < nwc:
                    emit_l1(w)
                r = w - LOOKAHEAD
                if 0 <= r < nwc:
                    emit_reduce(r)
                    if r % GW == GW - 1 or r == nwc - 1:
                        g0 = (r // GW) * GW
                        gn = r - g0 + 1
                        dly = 5 if g0 == 0 else 0
                        for k, fn in TAIL_PRE.items():
                            pre_at.setdefault(w + k + dly, []).append(
                                (fn, g0, gn))
                        for k, fn in TAIL_POST.items():
                            post_at.setdefault(w + k + dly, []).append(
                                (fn, g0, gn))
                for fn, g0, gn in post_at.pop(w, []):
                    fn(g0, gn)

    nc.compile()
    return nc


def _rho0(wts):
    """rho_mlp(0) for empty segments (host fp32)."""
    r1 = np.maximum(np.asarray(wts["rho_b1"], np.float64), 0.0)
    return (r1 @ np.asarray(wts["rho_W2"], np.float64)
            + np.asarray(wts["rho_b2"], np.float64)).astype(np.float32)


def _run(inputs, trace=False, **hw_kwargs):
    ins = np.asarray(inputs["ins"])
    batch = np.asarray(inputs["batch"])
    per_core, nwc, col_of_seg, counts = _host_prep(ins, batch, inputs)
    consts_np = _host_consts(inputs)
    nc = _build(nwc, consts_np)

    in_maps = []
    for c in range(N_CORES):
        m = dict(consts_np)
        m.update(per_core[c])
        in_maps.append(m)
    res = run_bass_kernel_spmd(
        nc, in_maps, core_ids=list(range(N_CORES)), trace=trace, **hw_kwargs
    )
    outs = [np.asarray(r["outT"], np.float32) for r in res.results]
    allc = np.concatenate(outs, axis=1)          # [128, n_win*64]
    full = np.empty((NSEG, P), np.float32)
    occ = counts > 0
    full[occ] = allc[:, col_of_seg[occ]].T
    if (~occ).any():
        full[~occ] = _rho0(inputs)
    return full, res


def kernel(**inputs):
    out, _ = _run(inputs)
    return out


# revision 53
# speedup vs baseline: 1.2681x; 1.0265x over previous
"""DeepSets segment-reduce kernel for 8x TRN2 NeuronCores (Bass/Tile).

Computes: out = rho_mlp(segment_mean(phi_mlp(ins), batch))  for
sorted segment ids `batch` in [0, 50000), ins [1M, 128] f32.

v3 design (vs the padded-window baseline):
  - Host REBALANCES segments into windows of <=SEGS_W segs AND <=ROWS_W
    rows (first-fit-decreasing), so every window is exactly NB 128-row
    blocks with ~1.5% padding (baseline: 10%) and every core runs an
    IDENTICAL instruction stream (SPMD-safe static shapes).
  - The one-hot scatter matrix S is [128, 64] per block (segment slot
    within the window), 4x smaller than the baseline's [128, 128], and
    the reduce matmul streams only 64 columns.
  - 1/count is NOT folded into X. It is applied at the PSUM drain of the
    segment sums: t_sb = tps * invc128 (DVE tensor_tensor mult), which
    replaces the plain copy at zero extra cost. X carries only +u0
    (exact phi-b1 absorption: u0 = W1f^-T b1).
  - phi_b2 / rho biases are applied as per-partition ACT biases in the
    feature-major tail; the b2*nz masking matmul is gone: empty segments
    are fixed up on the host (out[empty] = rho_mlp(0)).
  - fp16 output DMA (host converts to fp32).

kernel(**inputs) takes the full unsharded inputs and returns the full
[50000, 128] fp32 output.
"""

import numpy as np
import ml_dtypes

import concourse.mybir as mybir
import concourse.tile as tile
from concourse import bacc
from concourse.bass_utils import run_bass_kernel_spmd

P = 128
N_CORES = 8
NSEG = 50000
SEGS_W = 64          # max segments per window (= S columns, psum slot width)
ROWS_W = 1024        # max rows per window = NB * 128
NB = ROWS_W // P     # blocks per window (8)
GW = 8               # windows per tail group (8*64 = 512 psum cols)
F16 = mybir.dt.float16
F32 = mybir.dt.float32
F8 = mybir.dt.float8e4
FP8NP = ml_dtypes.float8_e4m3

H1R_DT = F16         # h1r SBUF dtype (pocket: mybir.dt.float8e3)
H1R_NP = np.float16


def _f16(a):
    return np.asarray(a, dtype=np.float32).astype(np.float16)


def _pack_windows(counts):
    """First-fit-decreasing: segments -> windows of <=SEGS_W segs,
    <=ROWS_W rows. Returns (n_win, win_of_seg, windows list of seg lists)."""
    nseg = len(counts)
    occupied = np.flatnonzero(counts > 0)
    order = occupied[np.argsort(-counts[occupied], kind="stable")]
    total = int(counts.sum())
    n_min = max(-(-total // ROWS_W), -(-len(occupied) // SEGS_W))
    n_win = -(-(n_min + max(2, n_min // 120)) // N_CORES) * N_CORES
    while True:
        wrows = np.zeros(n_win, np.int64)
        wsegs = np.zeros(n_win, np.int64)
        win_of = np.full(nseg, -1, np.int64)
        avail = list(range(n_win))
        ptr = 0
        ok = True
        for s in order:
            c = counts[s]
            placed = False
            for t in range(len(avail)):
                wi = avail[(ptr + t) % len(avail)]
                if wrows[wi] + c <= ROWS_W and wsegs[wi] < SEGS_W:
                    wrows[wi] += c
                    wsegs[wi] += 1
                    win_of[s] = wi
                    ptr = (ptr + t) % len(avail)
                    if wrows[wi] > ROWS_W - 3 or wsegs[wi] >= SEGS_W:
                        avail.remove(wi)
                        ptr = ptr % max(1, len(avail))
                    else:
                        ptr = (ptr + 1) % len(avail)
                    placed = True
                    break
            if not placed or not avail:
                ok = placed
                break
        if ok:
            return n_win, win_of
        n_win += N_CORES


def _host_prep(ins, batch, wts):
    batch = np.asarray(batch).astype(np.int64)
    ins = np.asarray(ins, dtype=np.float32)
    counts = np.bincount(batch, minlength=NSEG)
    seg_start = np.zeros(NSEG + 1, np.int64)
    np.cumsum(counts, out=seg_start[1:])

    # exact phi-b1 absorption: (x + u0) @ W1f == x @ W1f + b1
    W1f = _f16(wts["phi_W1"]).astype(np.float64)
    b1d = np.asarray(wts["phi_b1"], np.float64)
    u0 = np.linalg.solve(W1f.T, b1d)
    assert np.isfinite(u0).all() and np.abs(u0).max() < 64.0
    ins_u = (ins + u0.astype(np.float32)).astype(np.float16)
    ins_8 = np.clip(ins, -15.0, 15.0).astype(ml_dtypes.float8_e3m4)

    n_win, win_of = _pack_windows(counts)
    nwc = n_win // N_CORES

    # segment -> slot within window: order segs by (window, seg id)
    occ = np.flatnonzero(counts > 0)
    ow = win_of[occ]
    order2 = np.lexsort((occ, ow))
    segs_sorted = occ[order2]
    wins_sorted = ow[order2]
    first = np.r_[True, wins_sorted[1:] != wins_sorted[:-1]]
    slot = np.arange(len(segs_sorted)) - np.maximum.accumulate(
        np.where(first, np.arange(len(segs_sorted)), 0))
    slot_of = np.full(NSEG, -1, np.int64)
    slot_of[segs_sorted] = slot

    invc = np.zeros(NSEG, np.float64)
    invc[occ] = 1.0 / counts[occ]

    ncols = nwc * ROWS_W                        # xt columns per core
    nblk = nwc * NB
    per_core = []
    for c in range(N_CORES):
        rows_pad = np.full(ncols, -1, np.int64)
        m = (wins_sorted >= c * nwc) & (wins_sorted < (c + 1) * nwc)
        segs_c = segs_sorted[m]
        wins_c = wins_sorted[m] - c * nwc
        # rows of each seg are contiguous [seg_start[s], seg_start[s+1])
        cnt_c = counts[segs_c]
        woff = wins_c * ROWS_W
        # in-window offset = cumsum of counts within the window
        ccs = np.cumsum(cnt_c) - cnt_c
        wstart = np.zeros(len(segs_c), np.int64)
        if len(segs_c):
            fw = np.r_[True, wins_c[1:] != wins_c[:-1]]
            base = np.where(fw, ccs, 0)
            wstart = ccs - np.maximum.accumulate(base)
        dst0 = woff + wstart                    # first xt col of each seg
        starts = seg_start[segs_c]
        flat_dst = np.repeat(dst0, cnt_c) + (
            np.arange(int(cnt_c.sum()))
            - np.repeat(np.cumsum(cnt_c) - cnt_c, cnt_c))
        flat_src = np.repeat(starts, cnt_c) + (
            np.arange(int(cnt_c.sum()))
            - np.repeat(np.cumsum(cnt_c) - cnt_c, cnt_c))
        rows_pad[flat_dst] = flat_src

        valid = rows_pad >= 0
        # chunk0 (block cols [0,512) of each window) -> fp8 e3m4, no u0
        # (shifted relu on DVE + rank-1 b1*cdve fix); chunk1 -> fp16 + u0
        half = ROWS_W // 2
        cw = np.arange(ncols) % ROWS_W
        c0 = cw < half
        hidx = (np.arange(ncols) // ROWS_W) * half + cw % half
        x8b = np.zeros((nwc * half, P), ml_dtypes.float8_e3m4)
        v0 = valid & c0
        x8b[hidx[v0]] = ins_8[rows_pad[v0]]
        xt8 = np.ascontiguousarray(x8b.T)
        x16b = np.zeros((nwc * half, P), np.float16)
        v1 = valid & ~c0
        x16b[hidx[v1]] = ins_u[rows_pad[v1]]
        xt16 = np.ascontiguousarray(x16b.T)
        # rows-per-slot counts for the fp8 (shifted-relu) half
        slotidx = np.zeros(ncols, np.int64)
        slotidx[valid] = ((np.arange(ncols)[valid] // ROWS_W) * SEGS_W
                          + slot_of[batch[rows_pad[valid]]])
        cdve = np.bincount(slotidx[v0], minlength=nwc * SEGS_W)
        cdve = cdve.astype(np.float16).reshape(1, -1)

        # one-hot S fp8 [128, nblk*64]
        s8 = np.zeros((P, nblk * SEGS_W), np.uint8)
        ci = np.flatnonzero(valid)
        segcol = slot_of[batch[rows_pad[ci]]]
        blk = ci // P
        s8[ci % P, blk * SEGS_W + segcol] = 1
        sfp8 = s8.astype(FP8NP)

        # invc128 fp16 [128, nwc*64] (replicated across partitions)
        iv = np.zeros(nwc * SEGS_W, np.float32)
        iv[wins_c * SEGS_W + slot_of[segs_c]] = invc[segs_c].astype(np.float32)
        invc128 = np.ascontiguousarray(
            np.broadcast_to(iv.astype(np.float16), (P, nwc * SEGS_W)))

        per_core.append({"xt8": xt8, "xt16": xt16, "sfp8": sfp8,
                         "invc128": invc128, "cdve": cdve})

    # output slot -> segment mapping
    col_of_seg = np.full(NSEG, -1, np.int64)
    col_of_seg[segs_sorted] = wins_sorted * SEGS_W + slot_of[segs_sorted]
    return per_core, nwc, col_of_seg, counts


def _host_consts(wts):
    cpack16 = np.concatenate(
        [_f16(wts["phi_W1"]), _f16(wts["phi_W2"]),
         _f16(wts["rho_W1"]), _f16(wts["rho_W2"])], axis=1)
    cpack32 = np.stack(
        [np.asarray(wts["phi_b2"], np.float32),
         np.asarray(wts["rho_b1"], np.float32),
         np.asarray(wts["rho_b2"], np.float32)], axis=1)
    b1f = _f16(wts["phi_b1"])
    negb1t = np.ascontiguousarray(np.tile(-b1f, (P, 4)))
    b1row = b1f.reshape(1, P)
    return {"cpack16": cpack16, "cpack32": cpack32,
            "negb1t": negb1t, "b1row": b1row}


def _build(nwc, consts_np):
    """Emit the SPMD single-core program (same NEFF for all 8 cores)."""
    nblk = nwc * NB
    nc = bacc.Bacc("TRN2", target_bir_lowering=False, debug=False,
                   num_devices=N_CORES)

    half = ROWS_W // 2
    d_xt8 = nc.dram_tensor("xt8", [P, nwc * half], mybir.dt.float8e3,
                           kind="ExternalInput").ap()
    d_xt16 = nc.dram_tensor("xt16", [P, nwc * half], F16,
                            kind="ExternalInput").ap()
    d_cd = nc.dram_tensor("cdve", [1, nwc * SEGS_W], F16,
                          kind="ExternalInput").ap()
    d_s = nc.dram_tensor("sfp8", [P, nblk * SEGS_W], F8,
                         kind="ExternalInput").ap()
    d_iv = nc.dram_tensor("invc128", [P, nwc * SEGS_W], F16,
                          kind="ExternalInput").ap()
    d_consts = {
        k: nc.dram_tensor(
            k, list(v.shape), mybir.dt.from_np(v.dtype), kind="ExternalInput"
        ).ap()
        for k, v in consts_np.items()
    }
    d_out = nc.dram_tensor("outT", [P, nwc * SEGS_W], F16,
                           kind="ExternalOutput").ap()

    groups = []
    w0 = 0
    while w0 < nwc:
        groups.append((w0, min(GW, nwc - w0)))
        w0 += GW

    with tile.TileContext(nc) as tc:
        with (
            tc.tile_pool(name="const", bufs=1) as constp,
            tc.tile_pool(name="outsb", bufs=1) as outp,
            tc.tile_pool(name="xt", bufs=9) as xtp,
            tc.tile_pool(name="xt8", bufs=9) as xtp8,
            tc.tile_pool(name="sfp", bufs=3) as sfpp,
            tc.tile_pool(name="h1r", bufs=5) as h1rp,
            tc.tile_pool(name="tail16", bufs=6) as tailp,
            tc.tile_pool(name="h1ps", bufs=4, space="PSUM") as h1psp,
            tc.tile_pool(name="tps", bufs=2, space="PSUM") as tpsp,
            tc.tile_pool(name="tailps", bufs=2, space="PSUM") as tailpsp,
        ):
            cs_ = {}
            for k, v in consts_np.items():
                cs_[k] = constp.tile(
                    list(v.shape), mybir.dt.from_np(v.dtype), name=f"c_{k}")
                nc.scalar.dma_start(cs_[k], d_consts[k])
            w1_c = cs_["cpack16"][:, 0:128]
            w2_c = cs_["cpack16"][:, 128:256]
            rw1_c = cs_["cpack16"][:, 256:384]
            rw2_c = cs_["cpack16"][:, 384:512]
            b2_c = cs_["cpack32"][:, 0:1]
            rb1_c = cs_["cpack32"][:, 1:2]
            rb2_c = cs_["cpack32"][:, 2:3]
            # ivsb (2MB) goes on the striped sync ring, issued lazily at
            # window 3 so it doesn't delay the startup xt/S quarters and
            # doesn't serialize on the scalar ring's single queue
            ivsb = constp.tile([P, nwc * SEGS_W], F16, name="ivsb")
            cdsb = constp.tile([1, nwc * SEGS_W], F16, name="cdsb")
            nc.scalar.dma_start(cdsb, d_cd)
            negb1_c = cs_["negb1t"]
            b1r_c = cs_["b1row"]
            outsb = outp.tile([P, nwc * SEGS_W], F16)

            # DMA granularity: xt per XQ windows, S per SQ windows
            # (both 2-4KB/partition lines)
            XQ, SQ = 2, 8
            XW = XQ * half
            SW = SQ * NB * SEGS_W
            xt_tiles, s_tiles = {}, {}

            def fetch_xt(w):
                q = w // XQ
                if q in xt_tiles:
                    return
                t8 = xtp8.tile([P, XW], mybir.dt.float8e3, tag="xt8",
                               name=f"xt8_{q}")
                t16 = xtp.tile([P, XW], F16, tag="xt", name=f"xt16_{q}")
                lo = q * XW
                hi = min((q + 1) * XW, nwc * half)
                nc.sync.dma_start(t8[:, : hi - lo], d_xt8[:, lo:hi])
                nc.sync.dma_start(t16[:, : hi - lo], d_xt16[:, lo:hi])
                xt_tiles[q] = (t8, t16)

            def fetch_s(w):
                q = w // SQ
                if q in s_tiles:
                    return
                t = sfpp.tile([P, SW], F8, tag="sfp", name=f"sfp_{q}")
                lo = q * SW
                hi = min((q + 1) * SW, nblk * SEGS_W)
                wS = NB * SEGS_W
                if q == 0:
                    # split so the first window's reduce can start sooner
                    nc.sync.dma_start(t[:, :wS], d_s[:, :wS])
                    nc.sync.dma_start(t[:, wS: hi - lo], d_s[:, wS:hi])
                else:
                    nc.sync.dma_start(t[:, : hi - lo], d_s[:, lo:hi])
                s_tiles[q] = t

            h1r_tiles = {}
            tps_tiles = {}

            PF = 14   # DMA prefetch depth in windows

            def emit_l1(w):
                """xt DMA + phi layer 1 + relu drain for window w."""
                for pw in range(w, min(w + PF + 1, nwc)):
                    fetch_xt(pw)
                    fetch_s(pw)
                t8, t16 = xt_tiles[w // XQ]
                xoff = (w % XQ) * half
                h1r = h1rp.tile([P, ROWS_W], H1R_DT, tag="h1r", name=f"h1r_{w}")
                for cj in range(2):
                    xt = t8 if cj == 0 else t16
                    h1ps = h1psp.tile([P, 512], F32, space="PSUM",
                                      tag="h1ps")
                    for j in range(4):
                        nc.tensor.matmul(
                            h1ps[:, j * P: (j + 1) * P],
                            lhsT=xt[:, xoff + j * P: xoff + (j + 1) * P],
                            rhs=w1_c, start=True, stop=(j == 3),
                        )
                    dst = h1r[:, cj * 512: (cj + 1) * 512]
                    if cj == 0:
                        # fp8 half: shifted relu max(z, -b1); the missing
                        # +b1 is restored by the group's rank-1 b1*cdve
                        nc.vector.tensor_tensor(
                            dst, h1ps, negb1_c, mybir.AluOpType.max)
                    else:
                        nc.scalar.activation(
                            dst, h1ps, mybir.ActivationFunctionType.Relu)
                h1r_tiles[w] = h1r

            def emit_reduce(w):
                """segment-sum reduce of window w into its group psum."""
                g = w // GW
                gn_g = min(GW, nwc - g * GW)
                if g not in tps_tiles:
                    tps_tiles[g] = tpsp.tile(
                        [P, GW * SEGS_W], F32, space="PSUM", tag="tps",
                        name=f"tps_{g}")
                    # rank-1 b1 (x) cdve opens the accumulation: restores
                    # the b1 omitted by the fp8 half's shifted relu
                    nc.tensor.matmul(
                        tps_tiles[g][:, : gn_g * SEGS_W],
                        lhsT=b1r_c,
                        rhs=cdsb[:, g * GW * SEGS_W:
                                 (g * GW + gn_g) * SEGS_W],
                        start=True, stop=False,
                    )
                tps = tps_tiles[g]
                st = s_tiles[w // SQ]
                soff = (w % SQ) * NB * SEGS_W
                h1r = h1r_tiles.pop(w)
                reg = tps[:, (w % GW) * SEGS_W: (w % GW + 1) * SEGS_W]
                last_w = (w % GW == GW - 1) or (w == nwc - 1)
                for b in range(NB):
                    nc.tensor.matmul(
                        reg,
                        lhsT=h1r[:, b * P: (b + 1) * P],
                        rhs=st[:, soff + b * SEGS_W:
                               soff + (b + 1) * SEGS_W],
                        start=False, stop=(last_w and b == NB - 1),
                    )

            # staged tail: copies go at window START (ahead of relu in the
            # strict ACT/DVE FIFOs), matmuls at window END (after reduce),
            # each op a full window after its input was produced
            tail_state = {}

            def t_tsb(g0, gn):
                tps = tps_tiles.pop(g0 // GW)
                gc = gn * SEGS_W
                t_sb = tailp.tile([P, gc], F16, tag="t_sb",
                                  padded_shape=[P, GW * SEGS_W],
                                  name=f"t_sb_{g0}")
                nc.vector.tensor_tensor(
                    t_sb, tps[:, :gc],
                    ivsb[:, g0 * SEGS_W: g0 * SEGS_W + gc],
                    mybir.AluOpType.mult)
                tail_state[g0] = t_sb

            def t_smmm(g0, gn):
                gc = gn * SEGS_W
                smps = tailpsp.tile([P, gc], F32, space="PSUM",
                                    tag="tailps",
                                    padded_shape=[P, GW * SEGS_W],
                                    name=f"smps_{g0}")
                nc.tensor.matmul(smps, lhsT=w2_c, rhs=tail_state.pop(g0),
                                 start=True, stop=True)
                tail_state[g0] = smps

            def t_smcp(g0, gn):
                gc = gn * SEGS_W
                sm_sb = tailp.tile([P, gc], F16, tag="sm_sb",
                                   padded_shape=[P, GW * SEGS_W],
                                   name=f"sm_sb_{g0}")
                nc.scalar.activation(
                    sm_sb, tail_state.pop(g0),
                    mybir.ActivationFunctionType.Identity, bias=b2_c)
                tail_state[g0] = sm_sb

            def t_r1mm(g0, gn):
                gc = gn * SEGS_W
                r1ps = tailpsp.tile([P, gc], F32, space="PSUM",
                                    tag="tailps",
                                    padded_shape=[P, GW * SEGS_W],
                                    name=f"r1ps_{g0}")
                nc.tensor.matmul(r1ps, lhsT=rw1_c, rhs=tail_state.pop(g0),
                                 start=True, stop=True)
                tail_state[g0] = r1ps

            def t_r1cp(g0, gn):
                gc = gn * SEGS_W
                r1_sb = tailp.tile([P, gc], F16, tag="r1_sb",
                                   padded_shape=[P, GW * SEGS_W],
                                   name=f"r1_sb_{g0}")
                nc.vector.tensor_scalar(
                    r1_sb, tail_state.pop(g0), rb1_c, 0.0,
                    op0=mybir.AluOpType.add, op1=mybir.AluOpType.max)
                tail_state[g0] = r1_sb

            def t_opmm(g0, gn):
                gc = gn * SEGS_W
                ops_ = tailpsp.tile([P, gc], F32, space="PSUM",
                                    tag="tailps",
                                    padded_shape=[P, GW * SEGS_W],
                                    name=f"ops_{g0}")
                nc.tensor.matmul(ops_, lhsT=rw2_c, rhs=tail_state.pop(g0),
                                 start=True, stop=True)
                tail_state[g0] = ops_

            def t_outcp(g0, gn):
                gc = gn * SEGS_W
                nc.scalar.activation(
                    outsb[:, g0 * SEGS_W: g0 * SEGS_W + gc],
                    tail_state.pop(g0),
                    mybir.ActivationFunctionType.Identity, bias=rb2_c)
                nc.sync.dma_start(
                    d_out[:, g0 * SEGS_W: g0 * SEGS_W + gc],
                    outsb[:, g0 * SEGS_W: g0 * SEGS_W + gc])

            TAIL_PRE = {1: t_tsb, 3: t_smcp, 5: t_r1cp, 7: t_outcp}
            TAIL_POST = {2: t_smmm, 4: t_r1mm, 6: t_opmm}

            # software pipeline: L1 runs LOOKAHEAD windows ahead of the
            # reduce so the relu drains never stall the PE queue
            LOOKAHEAD = 2
            pre_at, post_at = {}, {}
            for w in range(nwc + LOOKAHEAD + 8):
                if w == 3:
                    nc.sync.dma_start(ivsb, d_iv)
                for fn, g0, gn in pre_at.pop(w, []):
                    fn(g0, gn)
                if w < nwc:
                    emit_l1(w)
                r = w - LOOKAHEAD
                if 0 <= r < nwc:
                    emit_reduce(r)
                    if r % GW == GW - 1 or r == nwc - 1:
                        g0 = (r // GW) * GW
                        gn = r - g0 + 1
                        dly = 5 if g0 == 0 else 0
                        for k, fn in TAIL_PRE.items():
                            pre_at.setdefault(w + k + dly, []).append(
                                (fn, g0, gn))
                        for k, fn in TAIL_POST.items():
                            post_at.setdefault(w + k + dly, []).append(
                                (fn, g0, gn))
                for fn, g0, gn in post_at.pop(w, []):
                    fn(g0, gn)

    nc.compile()
    return nc


def _rho0(wts):
    """rho_mlp(0) for empty segments (host fp32)."""
    r1 = np.maximum(np.asarray(wts["rho_b1"], np.float64), 0.0)
    return (r1 @ np.asarray(wts["rho_W2"], np.float64)
            + np.asarray(wts["rho_b2"], np.float64)).astype(np.float32)


def _run(inputs, trace=False, **hw_kwargs):
    ins = np.asarray(inputs["ins"])
    batch = np.asarray(inputs["batch"])
    per_core, nwc, col_of_seg, counts = _host_prep(ins, batch, inputs)
    consts_np = _host_consts(inputs)
    nc = _build(nwc, consts_np)

    in_maps = []
    for c in range(N_CORES):
        m = dict(consts_np)
        m.update(per_core[c])
        in_maps.append(m)
    res = run_bass_kernel_spmd(
        nc, in_maps, core_ids=list(range(N_CORES)), trace=trace, **hw_kwargs
    )
    outs = [np.asarray(r["outT"], np.float32) for r in res.results]
    allc = np.concatenate(outs, axis=1)          # [128, n_win*64]
    full = np.empty((NSEG, P), np.float32)
    occ = counts > 0
    full[occ] = allc[:, col_of_seg[occ]].T
    if (~occ).any():
        full[~occ] = _rho0(inputs)
    return full, res


def kernel(**inputs):
    out, _ = _run(inputs)
    return out


# revision 55
# speedup vs baseline: 1.2852x; 1.0136x over previous
"""DeepSets segment-reduce kernel for 8x TRN2 NeuronCores (Bass/Tile).

Computes: out = rho_mlp(segment_mean(phi_mlp(ins), batch))  for
sorted segment ids `batch` in [0, 50000), ins [1M, 128] f32.

v3 design (vs the padded-window baseline):
  - Host REBALANCES segments into windows of <=SEGS_W segs AND <=ROWS_W
    rows (first-fit-decreasing), so every window is exactly NB 128-row
    blocks with ~1.5% padding (baseline: 10%) and every core runs an
    IDENTICAL instruction stream (SPMD-safe static shapes).
  - The one-hot scatter matrix S is [128, 64] per block (segment slot
    within the window), 4x smaller than the baseline's [128, 128], and
    the reduce matmul streams only 64 columns.
  - 1/count is NOT folded into X. It is applied at the PSUM drain of the
    segment sums: t_sb = tps * invc128 (DVE tensor_tensor mult), which
    replaces the plain copy at zero extra cost. X carries only +u0
    (exact phi-b1 absorption: u0 = W1f^-T b1).
  - phi_b2 / rho biases are applied as per-partition ACT biases in the
    feature-major tail; the b2*nz masking matmul is gone: empty segments
    are fixed up on the host (out[empty] = rho_mlp(0)).
  - fp16 output DMA (host converts to fp32).

kernel(**inputs) takes the full unsharded inputs and returns the full
[50000, 128] fp32 output.
"""

import numpy as np
import ml_dtypes

import concourse.mybir as mybir
import concourse.tile as tile
from concourse import bacc
from concourse.bass_utils import run_bass_kernel_spmd

P = 128
N_CORES = 8
NSEG = 50000
SEGS_W = 64          # max segments per window (= S columns, psum slot width)
ROWS_W = 1024        # max rows per window = NB * 128
NB = ROWS_W // P     # blocks per window (8)
GW = 8               # windows per tail group (8*64 = 512 psum cols)
F16 = mybir.dt.float16
F32 = mybir.dt.float32
F8 = mybir.dt.float8e4
FP8NP = ml_dtypes.float8_e4m3

H1R_DT = F16         # h1r SBUF dtype (pocket: mybir.dt.float8e3)
H1R_NP = np.float16


def _f16(a):
    return np.asarray(a, dtype=np.float32).astype(np.float16)


def _pack_windows(counts):
    """First-fit-decreasing: segments -> windows of <=SEGS_W segs,
    <=ROWS_W rows. Returns (n_win, win_of_seg, windows list of seg lists)."""
    nseg = len(counts)
    occupied = np.flatnonzero(counts > 0)
    order = occupied[np.argsort(-counts[occupied], kind="stable")]
    total = int(counts.sum())
    n_min = max(-(-total // ROWS_W), -(-len(occupied) // SEGS_W))
    n_win = -(-(n_min + max(2, n_min // 120)) // N_CORES) * N_CORES
    while True:
        wrows = np.zeros(n_win, np.int64)
        wsegs = np.zeros(n_win, np.int64)
        win_of = np.full(nseg, -1, np.int64)
        avail = list(range(n_win))
        ptr = 0
        ok = True
        for s in order:
            c = counts[s]
            placed = False
            for t in range(len(avail)):
                wi = avail[(ptr + t) % len(avail)]
                if wrows[wi] + c <= ROWS_W and wsegs[wi] < SEGS_W:
                    wrows[wi] += c
                    wsegs[wi] += 1
                    win_of[s] = wi
                    ptr = (ptr + t) % len(avail)
                    if wrows[wi] > ROWS_W - 3 or wsegs[wi] >= SEGS_W:
                        avail.remove(wi)
                        ptr = ptr % max(1, len(avail))
                    else:
                        ptr = (ptr + 1) % len(avail)
                    placed = True
                    break
            if not placed or not avail:
                ok = placed
                break
        if ok:
            return n_win, win_of
        n_win += N_CORES


def _host_prep(ins, batch, wts):
    batch = np.asarray(batch).astype(np.int64)
    ins = np.asarray(ins, dtype=np.float32)
    counts = np.bincount(batch, minlength=NSEG)
    seg_start = np.zeros(NSEG + 1, np.int64)
    np.cumsum(counts, out=seg_start[1:])

    # exact phi-b1 absorption: (x + u0) @ W1f == x @ W1f + b1
    W1f = _f16(wts["phi_W1"]).astype(np.float64)
    b1d = np.asarray(wts["phi_b1"], np.float64)
    u0 = np.linalg.solve(W1f.T, b1d)
    assert np.isfinite(u0).all() and np.abs(u0).max() < 64.0
    ins_u = (ins + u0.astype(np.float32)).astype(np.float16)
    ins_8 = np.clip(ins, -15.0, 15.0).astype(ml_dtypes.float8_e3m4)

    n_win, win_of = _pack_windows(counts)
    nwc = n_win // N_CORES

    # segment -> slot within window: order segs by (window, seg id)
    occ = np.flatnonzero(counts > 0)
    ow = win_of[occ]
    order2 = np.lexsort((occ, ow))
    segs_sorted = occ[order2]
    wins_sorted = ow[order2]
    first = np.r_[True, wins_sorted[1:] != wins_sorted[:-1]]
    slot = np.arange(len(segs_sorted)) - np.maximum.accumulate(
        np.where(first, np.arange(len(segs_sorted)), 0))
    slot_of = np.full(NSEG, -1, np.int64)
    slot_of[segs_sorted] = slot

    invc = np.zeros(NSEG, np.float64)
    invc[occ] = 1.0 / counts[occ]

    ncols = nwc * ROWS_W                        # xt columns per core
    nblk = nwc * NB
    per_core = []
    for c in range(N_CORES):
        rows_pad = np.full(ncols, -1, np.int64)
        m = (wins_sorted >= c * nwc) & (wins_sorted < (c + 1) * nwc)
        segs_c = segs_sorted[m]
        wins_c = wins_sorted[m] - c * nwc
        # rows of each seg are contiguous [seg_start[s], seg_start[s+1])
        cnt_c = counts[segs_c]
        woff = wins_c * ROWS_W
        # in-window offset = cumsum of counts within the window
        ccs = np.cumsum(cnt_c) - cnt_c
        wstart = np.zeros(len(segs_c), np.int64)
        if len(segs_c):
            fw = np.r_[True, wins_c[1:] != wins_c[:-1]]
            base = np.where(fw, ccs, 0)
            wstart = ccs - np.maximum.accumulate(base)
        dst0 = woff + wstart                    # first xt col of each seg
        starts = seg_start[segs_c]
        flat_dst = np.repeat(dst0, cnt_c) + (
            np.arange(int(cnt_c.sum()))
            - np.repeat(np.cumsum(cnt_c) - cnt_c, cnt_c))
        flat_src = np.repeat(starts, cnt_c) + (
            np.arange(int(cnt_c.sum()))
            - np.repeat(np.cumsum(cnt_c) - cnt_c, cnt_c))
        rows_pad[flat_dst] = flat_src

        valid = rows_pad >= 0
        # chunk0 (block cols [0,512) of each window) -> fp8 e3m4, no u0
        # (shifted relu on DVE + rank-1 b1*cdve fix); chunk1 -> fp16 + u0
        half = ROWS_W // 2
        cw = np.arange(ncols) % ROWS_W
        c0 = cw < half
        hidx = (np.arange(ncols) // ROWS_W) * half + cw % half
        x8b = np.zeros((nwc * half, P), ml_dtypes.float8_e3m4)
        v0 = valid & c0
        x8b[hidx[v0]] = ins_8[rows_pad[v0]]
        xt8 = np.ascontiguousarray(x8b.T)
        x16b = np.zeros((nwc * half, P), np.float16)
        v1 = valid & ~c0
        x16b[hidx[v1]] = ins_u[rows_pad[v1]]
        xt16 = np.ascontiguousarray(x16b.T)
        # rows-per-slot counts for the fp8 (shifted-relu) half
        slotidx = np.zeros(ncols, np.int64)
        slotidx[valid] = ((np.arange(ncols)[valid] // ROWS_W) * SEGS_W
                          + slot_of[batch[rows_pad[valid]]])
        cdve = np.bincount(slotidx[v0], minlength=nwc * SEGS_W)
        cdve = cdve.astype(np.float16).reshape(1, -1)

        # one-hot S fp8 [128, nblk*64]
        s8 = np.zeros((P, nblk * SEGS_W), np.uint8)
        ci = np.flatnonzero(valid)
        segcol = slot_of[batch[rows_pad[ci]]]
        blk = ci // P
        s8[ci % P, blk * SEGS_W + segcol] = 1
        sfp8 = s8.astype(FP8NP)

        # invc128 fp16 [128, nwc*64] (replicated across partitions)
        iv = np.zeros(nwc * SEGS_W, np.float32)
        iv[wins_c * SEGS_W + slot_of[segs_c]] = invc[segs_c].astype(np.float32)
        invc128 = np.ascontiguousarray(
            np.broadcast_to(iv.astype(np.float16), (P, nwc * SEGS_W)))

        per_core.append({"xt8": xt8, "xt16": xt16, "sfp8": sfp8,
                         "invc128": invc128, "cdve": cdve})

    # output slot -> segment mapping
    col_of_seg = np.full(NSEG, -1, np.int64)
    col_of_seg[segs_sorted] = wins_sorted * SEGS_W + slot_of[segs_sorted]
    return per_core, nwc, col_of_seg, counts


def _host_consts(wts):
    cpack16 = np.concatenate(
        [_f16(wts["phi_W1"]), _f16(wts["phi_W2"]),
         _f16(wts["rho_W1"]), _f16(wts["rho_W2"])], axis=1)
    cpack32 = np.stack(
        [np.asarray(wts["phi_b2"], np.float32),
         np.asarray(wts["rho_b1"], np.float32),
         np.asarray(wts["rho_b2"], np.float32)], axis=1)
    b1f = _f16(wts["phi_b1"])
    negb1t = np.ascontiguousarray(np.tile(-b1f, (P, 4)))
    b1row = b1f.reshape(1, P)
    return {"cpack16": cpack16, "cpack32": cpack32,
            "negb1t": negb1t, "b1row": b1row}


def _build(nwc, consts_np):
    """Emit the SPMD single-core program (same NEFF for all 8 cores)."""
    nblk = nwc * NB
    nc = bacc.Bacc("TRN2", target_bir_lowering=False, debug=False,
                   num_devices=N_CORES)

    half = ROWS_W // 2
    d_xt8 = nc.dram_tensor("xt8", [P, nwc * half], mybir.dt.float8e3,
                           kind="ExternalInput").ap()
    d_xt16 = nc.dram_tensor("xt16", [P, nwc * half], F16,
                            kind="ExternalInput").ap()
    d_cd = nc.dram_tensor("cdve", [1, nwc * SEGS_W], F16,
                          kind="ExternalInput").ap()
    d_s = nc.dram_tensor("sfp8", [P, nblk * SEGS_W], F8,
                         kind="ExternalInput").ap()
    d_iv = nc.dram_tensor("invc128", [P, nwc * SEGS_W], F16,
                          kind="ExternalInput").ap()
    d_consts = {
        k: nc.dram_tensor(
            k, list(v.shape), mybir.dt.from_np(v.dtype), kind="ExternalInput"
        ).ap()
        for k, v in consts_np.items()
    }
    d_out = nc.dram_tensor("outT", [P, nwc * SEGS_W], F16,
                           kind="ExternalOutput").ap()

    groups = []
    w0 = 0
    while w0 < nwc:
        groups.append((w0, min(GW, nwc - w0)))
        w0 += GW

    with tile.TileContext(nc) as tc:
        with (
            tc.tile_pool(name="const", bufs=1) as constp,
            tc.tile_pool(name="outsb", bufs=1) as outp,
            tc.tile_pool(name="xt", bufs=9) as xtp,
            tc.tile_pool(name="xt8", bufs=9) as xtp8,
            tc.tile_pool(name="sfp", bufs=3) as sfpp,
            tc.tile_pool(name="h1r", bufs=5) as h1rp,
            tc.tile_pool(name="tail16", bufs=6) as tailp,
            tc.tile_pool(name="h1ps", bufs=4, space="PSUM") as h1psp,
            tc.tile_pool(name="tps", bufs=2, space="PSUM") as tpsp,
            tc.tile_pool(name="tailps", bufs=2, space="PSUM") as tailpsp,
        ):
            cs_ = {}
            rings = {"cpack16": nc.scalar, "cpack32": nc.scalar,
                     "negb1t": nc.gpsimd, "b1row": nc.gpsimd}
            for k, v in consts_np.items():
                cs_[k] = constp.tile(
                    list(v.shape), mybir.dt.from_np(v.dtype), name=f"c_{k}")
                rings.get(k, nc.scalar).dma_start(cs_[k], d_consts[k])
            w1_c = cs_["cpack16"][:, 0:128]
            w2_c = cs_["cpack16"][:, 128:256]
            rw1_c = cs_["cpack16"][:, 256:384]
            rw2_c = cs_["cpack16"][:, 384:512]
            b2_c = cs_["cpack32"][:, 0:1]
            rb1_c = cs_["cpack32"][:, 1:2]
            rb2_c = cs_["cpack32"][:, 2:3]
            # ivsb (2MB) goes on the striped sync ring, issued lazily at
            # window 3 so it doesn't delay the startup xt/S quarters and
            # doesn't serialize on the scalar ring's single queue
            ivsb = constp.tile([P, nwc * SEGS_W], F16, name="ivsb")
            cdsb = constp.tile([1, nwc * SEGS_W], F16, name="cdsb")
            nc.gpsimd.dma_start(cdsb, d_cd)
            negb1_c = cs_["negb1t"]
            b1r_c = cs_["b1row"]
            outsb = outp.tile([P, nwc * SEGS_W], F16)

            # DMA granularity: xt per XQ windows, S per SQ windows
            # (both 2-4KB/partition lines)
            XQ, SQ = 2, 8
            XW = XQ * half
            SW = SQ * NB * SEGS_W
            xt_tiles, s_tiles = {}, {}

            def fetch_xt(w):
                q = w // XQ
                if q in xt_tiles:
                    return
                t8 = xtp8.tile([P, XW], mybir.dt.float8e3, tag="xt8",
                               name=f"xt8_{q}")
                t16 = xtp.tile([P, XW], F16, tag="xt", name=f"xt16_{q}")
                lo = q * XW
                hi = min((q + 1) * XW, nwc * half)
                nc.sync.dma_start(t8[:, : hi - lo], d_xt8[:, lo:hi])
                nc.sync.dma_start(t16[:, : hi - lo], d_xt16[:, lo:hi])
                xt_tiles[q] = (t8, t16)

            def fetch_s(w):
                q = w // SQ
                if q in s_tiles:
                    return
                t = sfpp.tile([P, SW], F8, tag="sfp", name=f"sfp_{q}")
                lo = q * SW
                hi = min((q + 1) * SW, nblk * SEGS_W)
                wS = NB * SEGS_W
                if q == 0:
                    # split so the first window's reduce can start sooner
                    nc.sync.dma_start(t[:, :wS], d_s[:, :wS])
                    nc.sync.dma_start(t[:, wS: hi - lo], d_s[:, wS:hi])
                else:
                    nc.sync.dma_start(t[:, : hi - lo], d_s[:, lo:hi])
                s_tiles[q] = t

            h1r_tiles = {}
            tps_tiles = {}

            PF = 14   # DMA prefetch depth in windows

            def emit_l1(w):
                """xt DMA + phi layer 1 + relu drain for window w."""
                for pw in range(w, min(w + PF + 1, nwc)):
                    fetch_xt(pw)
                    fetch_s(pw)
                t8, t16 = xt_tiles[w // XQ]
                xoff = (w % XQ) * half
                h1r = h1rp.tile([P, ROWS_W], H1R_DT, tag="h1r", name=f"h1r_{w}")
                for cj in range(2):
                    xt = t8 if cj == 0 else t16
                    h1ps = h1psp.tile([P, 512], F32, space="PSUM",
                                      tag="h1ps")
                    for j in range(4):
                        nc.tensor.matmul(
                            h1ps[:, j * P: (j + 1) * P],
                            lhsT=xt[:, xoff + j * P: xoff + (j + 1) * P],
                            rhs=w1_c, start=True, stop=(j == 3),
                        )
                    dst = h1r[:, cj * 512: (cj + 1) * 512]
                    if cj == 0:
                        # fp8 half: shifted relu max(z, -b1); the missing
                        # +b1 is restored by the group's rank-1 b1*cdve
                        nc.vector.tensor_tensor(
                            dst, h1ps, negb1_c, mybir.AluOpType.max)
                    else:
                        nc.scalar.activation(
                            dst, h1ps, mybir.ActivationFunctionType.Relu)
                h1r_tiles[w] = h1r

            def emit_reduce(w):
                """segment-sum reduce of window w into its group psum."""
                g = w // GW
                gn_g = min(GW, nwc - g * GW)
                if g not in tps_tiles:
                    tps_tiles[g] = tpsp.tile(
                        [P, GW * SEGS_W], F32, space="PSUM", tag="tps",
                        name=f"tps_{g}")
                    # rank-1 b1 (x) cdve opens the accumulation: restores
                    # the b1 omitted by the fp8 half's shifted relu
                    nc.tensor.matmul(
                        tps_tiles[g][:, : gn_g * SEGS_W],
                        lhsT=b1r_c,
                        rhs=cdsb[:, g * GW * SEGS_W:
                                 (g * GW + gn_g) * SEGS_W],
                        start=True, stop=False,
                    )
                tps = tps_tiles[g]
                st = s_tiles[w // SQ]
                soff = (w % SQ) * NB * SEGS_W
                h1r = h1r_tiles.pop(w)
                reg = tps[:, (w % GW) * SEGS_W: (w % GW + 1) * SEGS_W]
                last_w = (w % GW == GW - 1) or (w == nwc - 1)
                for b in range(NB):
                    nc.tensor.matmul(
                        reg,
                        lhsT=h1r[:, b * P: (b + 1) * P],
                        rhs=st[:, soff + b * SEGS_W:
                               soff + (b + 1) * SEGS_W],
                        start=False, stop=(last_w and b == NB - 1),
                    )

            # staged tail: copies go at window START (ahead of relu in the
            # strict ACT/DVE FIFOs), matmuls at window END (after reduce),
            # each op a full window after its input was produced
            tail_state = {}

            def t_tsb(g0, gn):
                tps = tps_tiles.pop(g0 // GW)
                gc = gn * SEGS_W
                t_sb = tailp.tile([P, gc], F16, tag="t_sb",
                                  padded_shape=[P, GW * SEGS_W],
                                  name=f"t_sb_{g0}")
                nc.vector.tensor_tensor(
                    t_sb, tps[:, :gc],
                    ivsb[:, g0 * SEGS_W: g0 * SEGS_W + gc],
                    mybir.AluOpType.mult)
                tail_state[g0] = t_sb

            def t_smmm(g0, gn):
                gc = gn * SEGS_W
                smps = tailpsp.tile([P, gc], F32, space="PSUM",
                                    tag="tailps",
                                    padded_shape=[P, GW * SEGS_W],
                                    name=f"smps_{g0}")
                nc.tensor.matmul(smps, lhsT=w2_c, rhs=tail_state.pop(g0),
                                 start=True, stop=True)
                tail_state[g0] = smps

            def t_smcp(g0, gn):
                gc = gn * SEGS_W
                sm_sb = tailp.tile([P, gc], F16, tag="sm_sb",
                                   padded_shape=[P, GW * SEGS_W],
                                   name=f"sm_sb_{g0}")
                nc.scalar.activation(
                    sm_sb, tail_state.pop(g0),
                    mybir.ActivationFunctionType.Identity, bias=b2_c)
                tail_state[g0] = sm_sb

            def t_r1mm(g0, gn):
                gc = gn * SEGS_W
                r1ps = tailpsp.tile([P, gc], F32, space="PSUM",
                                    tag="tailps",
                                    padded_shape=[P, GW * SEGS_W],
                                    name=f"r1ps_{g0}")
                nc.tensor.matmul(r1ps, lhsT=rw1_c, rhs=tail_state.pop(g0),
                                 start=True, stop=True)
                tail_state[g0] = r1ps

            def t_r1cp(g0, gn):
                gc = gn * SEGS_W
                r1_sb = tailp.tile([P, gc], F16, tag="r1_sb",
                                   padded_shape=[P, GW * SEGS_W],
                                   name=f"r1_sb_{g0}")
                nc.vector.tensor_scalar(
                    r1_sb, tail_state.pop(g0), rb1_c, 0.0,
                    op0=mybir.AluOpType.add, op1=mybir.AluOpType.max)
                tail_state[g0] = r1_sb

            def t_opmm(g0, gn):
                gc = gn * SEGS_W
                ops_ = tailpsp.tile([P, gc], F32, space="PSUM",
                                    tag="tailps",
                                    padded_shape=[P, GW * SEGS_W],
                                    name=f"ops_{g0}")
                nc.tensor.matmul(ops_, lhsT=rw2_c, rhs=tail_state.pop(g0),
                                 start=True, stop=True)
                tail_state[g0] = ops_

            def t_outcp(g0, gn):
                gc = gn * SEGS_W
                nc.scalar.activation(
                    outsb[:, g0 * SEGS_W: g0 * SEGS_W + gc],
                    tail_state.pop(g0),
                    mybir.ActivationFunctionType.Identity, bias=rb2_c)
                nc.sync.dma_start(
                    d_out[:, g0 * SEGS_W: g0 * SEGS_W + gc],
                    outsb[:, g0 * SEGS_W: g0 * SEGS_W + gc])

            TAIL_PRE = {1: t_tsb, 3: t_smcp, 5: t_r1cp, 7: t_outcp}
            TAIL_POST = {2: t_smmm, 4: t_r1mm, 6: t_opmm}

            # software pipeline: L1 runs LOOKAHEAD windows ahead of the
            # reduce so the relu drains never stall the PE queue
            LOOKAHEAD = 2
            pre_at, post_at = {}, {}
            for w in range(nwc + LOOKAHEAD + 8):
                if w == 3:
                    nc.sync.dma_start(ivsb, d_iv)
                for fn, g0, gn in pre_at.pop(w, []):
                    fn(g0, gn)
                if w < nwc:
                    emit_l1(w)
                r = w - LOOKAHEAD
                if 0 <= r < nwc:
                    emit_reduce(r)
                    if r % GW == GW - 1 or r == nwc - 1:
                        g0 = (r // GW) * GW
                        gn = r - g0 + 1
                        dly = 5 if g0 == 0 else 0
                        for k, fn in TAIL_PRE.items():
                            pre_at.setdefault(w + k + dly, []).append(
                                (fn, g0, gn))
                        for k, fn in TAIL_POST.items():
                            post_at.setdefault(w + k + dly, []).append(
                                (fn, g0, gn))
                for fn, g0, gn in post_at.pop(w, []):
                    fn(g0, gn)

    nc.compile()
    return nc


def _rho0(wts):
    """rho_mlp(0) for empty segments (host fp32)."""
    r1 = np.maximum(np.asarray(wts["rho_b1"], np.float64), 0.0)
    return (r1 @ np.asarray(wts["rho_W2"], np.float64)
            + np.asarray(wts["rho_b2"], np.float64)).astype(np.float32)


def _run(inputs, trace=False, **hw_kwargs):
    ins = np.asarray(inputs["ins"])
    batch = np.asarray(inputs["batch"])
    per_core, nwc, col_of_seg, counts = _host_prep(ins, batch, inputs)
    consts_np = _host_consts(inputs)
    nc = _build(nwc, consts_np)

    in_maps = []
    for c in range(N_CORES):
        m = dict(consts_np)
        m.update(per_core[c])
        in_maps.append(m)
    res = run_bass_kernel_spmd(
        nc, in_maps, core_ids=list(range(N_CORES)), trace=trace, **hw_kwargs
    )
    outs = [np.asarray(r["outT"], np.float32) for r in res.results]
    allc = np.concatenate(outs, axis=1)          # [128, n_win*64]
    full = np.empty((NSEG, P), np.float32)
    occ = counts > 0
    full[occ] = allc[:, col_of_seg[occ]].T
    if (~occ).any():
        full[~occ] = _rho0(inputs)
    return full, res


def kernel(**inputs):
    out, _ = _run(inputs)
    return out
